# revision 30
# baseline (speedup 1.0000x reference)
"""Segment-softmax feature aggregation (segment_reduce) for Trainium2.

Full inputs: x [8, 256, 128, 128] f32, preds [8, 19, 128, 128] f32.
Sharded batch-parallel across 8 NeuronCores (1 batch per core).

Per-core algorithm (B=1, C=256, N=16384 pixels, K=19 classes):
  s[n]    = max_k preds[k, n]           per-pixel max logit (f32, exact)
  mask    = (predsT == s)               one-hot argmax, bf16 (no ties in f32)
  wm      = mask * exp(s)               bf16 [n, k] tiles
  agg     = sum_n wm[n,:]^T (.) xaug[n,:]   PE bf16 matmul -> [k, C+2] f32 PSUM
            (xaug column C is 1.0 -> column C of agg is denom_k)
  aggN    = agg[:, :C] / denom          then cast bf16
  out     = aggN^T @ maskB              PE bf16 scatter matmul, maskB = mask^T
  out is stored bf16 [C, N] and upcast to f32 on the host.

Both x and preds are pre-transposed on the host into pixel-major layouts
(x also bf16-cast, padded with a ones column), so the device transposes
nothing except the tiny one-hot masks (batched 4 tiles per PE transpose).
DMAs are spread round-robin across three HWDGE queues (sync/gpsimd/vector
in, scalar/sync/gpsimd out) to overlap descriptor-generation latency.
"""

import numpy as np
import ml_dtypes

B, C, H, W, K = 8, 256, 128, 128, 19
N = H * W                  # 16384
TILE = 128                 # pixels per tile
NT = N // TILE             # 128 n-tiles
CP = C + 2                 # x row: 256 channels + ones col + pad col = 258
GA = 8                     # phase-A tiles per group
NGA = NT // GA             # 16 groups
OCH = 2048                 # out DMA chunk (free dim)
NCORES = 8

BF16 = ml_dtypes.bfloat16

_CACHE = {}


def _build_nc():
    import concourse.bacc as bacc
    import concourse.tile as tile
    from concourse import mybir

    f32 = mybir.dt.float32
    bf16 = mybir.dt.bfloat16
    i16 = mybir.dt.int16
    Alu = mybir.AluOpType
    Ax = mybir.AxisListType

    nc = bacc.Bacc("TRN2", target_bir_lowering=True)
    # x stored chunk-major: 8 chunks of [128 p, 16 tiles * 258], each chunk a
    # fully contiguous 1 MiB DRAM block (sequential HBM reads per DMA)
    x_d = nc.dram_tensor("x", [8 * TILE, 16 * CP], bf16, kind="ExternalInput")
    p_d = nc.dram_tensor("predsT", [TILE, NT * K], f32, kind="ExternalInput")
    # out chunk-major: 16 chunks of [128 c, 2048 n], contiguous per chunk
    o_d = nc.dram_tensor("out", [16 * 128, OCH], bf16, kind="ExternalOutput")

    with tile.TileContext(nc) as tc:
        with tc.tile_pool(name="singles", bufs=1) as singles:
            # bf16 identity for PE mask transposes, built on-device (no DMA)
            iot_r = singles.tile([128, 128], f32)
            nc.gpsimd.iota(
                iot_r, pattern=[[1, 128]], base=0, channel_multiplier=0,
                allow_small_or_imprecise_dtypes=True,
            )
            iot_c = singles.tile([128, 1], f32)
            nc.gpsimd.iota(
                iot_c, pattern=[[1, 1]], base=0, channel_multiplier=1,
                allow_small_or_imprecise_dtypes=True,
            )
            identb = singles.tile([128, 128], bf16)
            nc.vector.tensor_scalar(identb, iot_r, iot_c, None, Alu.is_equal)

            predsT_sb = singles.tile([TILE, NT * K], f32)
            nc.sync.dma_start(
                out=predsT_sb[:, : NT * K // 2], in_=p_d[:, : NT * K // 2]
            )
            nc.gpsimd.dma_start(
                out=predsT_sb[:, NT * K // 2:], in_=p_d[:, NT * K // 2:]
            )

            x_sb = singles.tile([TILE, NT * CP], bf16)
            in_q = [nc.sync, nc.gpsimd, nc.scalar]
            for q in range(8):                   # 8 x chunks of 16 tiles
                sl = slice(q * 16 * CP, (q + 1) * 16 * CP)
                in_q[q % 3].dma_start(
                    out=x_sb[:, sl], in_=x_d[q * TILE:(q + 1) * TILE, :]
                )

            s_all = singles.tile([128, NT], f32)
            w_all = singles.tile([128, NT], f32)
            # mask tiles padded to stride 32 so 4-tile PE transposes land at
            # 32-aligned PSUM partitions (engine partition-offset rule)
            maskA = singles.tile([128, NT * 32], bf16)
            wmA = singles.tile([128, NT * K], bf16)
            maskB = singles.tile([K, N], bf16)
            aggN = singles.tile([K, C], bf16)
            dinv = singles.tile([K, 1], f32)

            # ---- Phase A (DVE/ACT) + Phase B (PE) interleaved per group ----
            # group g = tiles 8g..8g+7; phase A computes s/w/maskA/wmA; PE
            # transposes masks (4 tiles per transpose) and accumulates agg.
            with (
                tc.tile_pool(name="psAgg", bufs=1, space="PSUM") as psAggp,
                tc.tile_pool(name="psMB", bufs=2, space="PSUM") as psMBp,
            ):
                psAgg = psAggp.tile([K, CP], f32)
                for g in range(NGA):
                    pg = predsT_sb[:, g * GA * K:(g + 1) * GA * K]
                    sg = s_all[:, g * GA:(g + 1) * GA]
                    nc.vector.tensor_reduce(
                        sg,
                        pg.rearrange("p (t k) -> p t k", k=K),
                        axis=Ax.X,
                        op=Alu.max,
                    )
                    nc.scalar.activation(
                        w_all[:, g * GA:(g + 1) * GA],
                        sg,
                        mybir.ActivationFunctionType.Exp,
                    )
                    mg = maskA.rearrange("p (t kk) -> p t kk", kk=32)[
                        :, g * GA:(g + 1) * GA, 0:K
                    ]
                    nc.vector.tensor_tensor(
                        mg,
                        pg.rearrange("p (t k) -> p t k", k=K),
                        sg[:, :, None].broadcast_to([128, GA, K]),
                        Alu.is_equal,
                    )
                    nc.vector.tensor_tensor(
                        wmA[:, g * GA * K:(g + 1) * GA * K].rearrange(
                            "p (t k) -> p t k", k=K
                        ),
                        mg,
                        w_all[:, g * GA:(g + 1) * GA][:, :, None].broadcast_to(
                            [128, GA, K]
                        ),
                        Alu.mult,
                    )
                    # PE: transpose this group's masks, 4 tiles per matmul.
                    # Batches of 8 transposes share one [76, 1024] PSUM tile
                    # (4 groups of 8 tiles = 32 tiles per PSUM tile).
                    if g % 4 == 0:
                        psMB = psMBp.tile([128, 8 * TILE], bf16)
                    for half in range(2):
                        j = g * 2 + half          # batch index (0..31)
                        jj = j % 8                # slot within psMB tile
                        nc.tensor.transpose(
                            psMB[:, jj * TILE:(jj + 1) * TILE],
                            maskA[:, j * 4 * 32:(j + 1) * 4 * 32],
                            identb,
                        )
                    if g % 4 == 3:
                        # evacuate 32 tiles of transposed masks -> maskB
                        g32 = g // 4
                        mbv = maskB.rearrange(
                            "k (g2 j r p) -> k g2 j r p", j=8, r=4, p=TILE
                        )
                        for r in range(4):
                            src = psMB[r * 32:r * 32 + K].rearrange(
                                "k (j p) -> k j p", p=TILE
                            )
                            if r % 2 == 0:
                                nc.vector.tensor_copy(mbv[:, g32, :, r, :], src)
                            else:
                                nc.scalar.copy(mbv[:, g32, :, r, :], src)
                    # PE: agg matmuls for this group's tiles
                    for t in range(g * GA, (g + 1) * GA):
                        nc.tensor.matmul(
                            psAgg,
                            lhsT=wmA[:, t * K:(t + 1) * K],
                            rhs=x_sb[:, t * CP:(t + 1) * CP],
                            start=(t == 0),
                            stop=(t == NT - 1),
                        )

                # ---- Phase C: normalize (reads psAgg in place) -------------
                nc.vector.tensor_scalar(
                    dinv, psAgg[:, C:C + 1], 1e-30, None, Alu.max
                )
                nc.vector.reciprocal(dinv, dinv)
                nc.vector.tensor_scalar(
                    aggN, psAgg[:, 0:C], dinv, None, Alu.mult
                )

            # ---- Phase D: scatter out = aggN^T @ maskB ---------------------
            with (
                tc.tile_pool(name="psO", bufs=3, space="PSUM") as psOp,
                tc.tile_pool(name="ost", bufs=4) as ostp,
            ):
                out_q = [nc.scalar, nc.sync, nc.gpsimd]
                ci = 0
                for h in range(2):
                    for q in range(N // OCH):          # OCH=2048, 8 per half
                        ost = ostp.tile([128, OCH], bf16)
                        for jh in range(OCH // 1024):
                            psO = psOp.tile([128, 1024], f32)
                            for j2 in range(2):
                                nb0 = q * OCH + jh * 1024 + j2 * 512
                                nc.tensor.matmul(
                                    psO[:, j2 * 512:(j2 + 1) * 512],
                                    lhsT=aggN[:, h * 128:(h + 1) * 128],
                                    rhs=maskB[:, nb0:nb0 + 512],
                                    start=True,
                                    stop=True,
                                )
                            dst = ost[:, jh * 1024:(jh + 1) * 1024]
                            if ci % 2 == 0:
                                nc.vector.tensor_copy(dst, psO)
                            else:
                                nc.scalar.copy(dst, psO)
                            ci += 1
                        oc = h * 8 + q
                        out_q[oc % 3].dma_start(
                            out=o_d[oc * 128:(oc + 1) * 128, :],
                            in_=ost,
                        )

    nc.compile()
    return nc


def _get_nc():
    if "nc" not in _CACHE:
        _CACHE["nc"] = _build_nc()
    return _CACHE["nc"]


def make_in_maps(x, preds):
    """Host-side marshaling: per batch, bf16-cast + transpose x into
    x_host[p, t, c] (c padded to 258 with [*chans, 1.0, 0.0]) and transpose
    preds into predsT_host[p, t, k] (f32, exact)."""
    x = np.asarray(x, dtype=np.float32)
    preds = np.asarray(preds, dtype=np.float32)
    in_maps = []
    for b in range(NCORES):
        xh = np.empty((TILE, NT, CP), dtype=BF16)
        # x[b]: [C, H, W] -> [C, NT, TILE]; want xh[p, t, c] = x[b][c, t, p]
        xh[:, :, :C] = x[b].reshape(C, NT, TILE).transpose(2, 1, 0)
        xh[:, :, C] = 1.0
        xh[:, :, C + 1] = 0.0
        pt = np.ascontiguousarray(
            preds[b].reshape(K, NT, TILE).transpose(2, 1, 0)
        )
        # chunk-major: [8 chunks, 128 p, 16*258] with each chunk contiguous
        xcm = np.ascontiguousarray(
            xh.reshape(TILE, 8, 16 * CP).transpose(1, 0, 2)
        )
        in_maps.append(
            {
                "x": xcm.reshape(8 * TILE, 16 * CP),
                "predsT": pt.reshape(TILE, NT * K),
            }
        )
    return in_maps


def kernel(x, preds):
    from concourse.bass_utils import run_bass_kernel_spmd

    nc = _get_nc()
    in_maps = make_in_maps(x, preds)
    res = run_bass_kernel_spmd(nc, in_maps, list(range(NCORES)))
    out = np.stack(
        [
            np.asarray(res.results[b]["out"])
            .astype(np.float32)
            .reshape(2, 8, 128, OCH)
            .transpose(0, 2, 1, 3)
            .reshape(C, H, W)
            for b in range(NCORES)
        ]
    )
    return out


# revision 36
# speedup vs baseline: 1.1701x; 1.1701x over previous
"""Segment-softmax feature aggregation (segment_reduce) for Trainium2.

Full inputs: x [8, 256, 128, 128] f32, preds [8, 19, 128, 128] f32.
Sharded batch-parallel across 8 NeuronCores (1 batch per core).

Per-core algorithm (B=1, C=256, N=16384 pixels, K=19 classes):
  s[n]    = max_k preds[k, n]           per-pixel max logit (f32, exact)
  mask    = (predsT == s)               one-hot argmax, bf16 (no ties in f32)
  wm      = mask * exp(s)               bf16 [n, k] tiles
  agg     = sum_n wm[n,:]^T (.) xaug[n,:]   PE bf16 matmul -> [k, C+2] f32 PSUM
            (xaug column C is 1.0 -> column C of agg is denom_k)
  aggN    = agg[:, :C] / denom          then cast bf16
  out     = aggN^T @ maskB              PE bf16 scatter matmul, maskB = mask^T
  out is stored bf16 [C, N] and upcast to f32 on the host.

Both x and preds are pre-transposed on the host into pixel-major layouts
(x also bf16-cast, padded with a ones column), so the device transposes
nothing except the tiny one-hot masks (batched 4 tiles per PE transpose).
DMAs are spread round-robin across three DMA-dispatch queues (sync/gpsimd/
scalar in, scalar/sync/gpsimd out) to overlap descriptor-generation latency.
"""

import numpy as np
import ml_dtypes

B, C, H, W, K = 8, 256, 128, 128, 19
N = H * W                  # 16384
TILE = 128                 # pixels per tile
NT = N // TILE             # 128 n-tiles
CP = C + 2                 # x row: 256 channels + ones col + pad col = 258
GA = 8                     # phase-A tiles per group
NGA = NT // GA             # 16 groups
OCH = 2048                 # out DMA chunk (free dim)
NCORES = 8

BF16 = ml_dtypes.bfloat16

_CACHE = {}


def _build_nc():
    import concourse.bacc as bacc
    import concourse.tile as tile
    from concourse import mybir

    f32 = mybir.dt.float32
    bf16 = mybir.dt.bfloat16
    i16 = mybir.dt.int16
    Alu = mybir.AluOpType
    Ax = mybir.AxisListType

    nc = bacc.Bacc("TRN2", target_bir_lowering=True)
    x_d = nc.dram_tensor("x", [TILE, NT * CP], bf16, kind="ExternalInput")
    p_d = nc.dram_tensor("predsT", [TILE, NT * K], f32, kind="ExternalInput")
    o_d = nc.dram_tensor("out", [C, N], bf16, kind="ExternalOutput")

    with tile.TileContext(nc) as tc:
        with tc.tile_pool(name="singles", bufs=1) as singles:
            # bf16 identity for PE mask transposes, built on-device (no DMA)
            iot_r = singles.tile([128, 128], f32)
            nc.gpsimd.iota(
                iot_r, pattern=[[1, 128]], base=0, channel_multiplier=0,
                allow_small_or_imprecise_dtypes=True,
            )
            iot_c = singles.tile([128, 1], f32)
            nc.gpsimd.iota(
                iot_c, pattern=[[1, 1]], base=0, channel_multiplier=1,
                allow_small_or_imprecise_dtypes=True,
            )
            identb = singles.tile([128, 128], bf16)
            nc.vector.tensor_scalar(identb, iot_r, iot_c, None, Alu.is_equal)

            predsT_sb = singles.tile([TILE, NT * K], f32)
            nc.sync.dma_start(
                out=predsT_sb[:, : NT * K // 2], in_=p_d[:, : NT * K // 2]
            )
            nc.gpsimd.dma_start(
                out=predsT_sb[:, NT * K // 2:], in_=p_d[:, NT * K // 2:]
            )

            x_sb = singles.tile([TILE, NT * CP], bf16)
            in_q = [nc.sync, nc.gpsimd, nc.scalar]
            for q in range(8):                   # 8 x chunks of 16 tiles
                sl = slice(q * 16 * CP, (q + 1) * 16 * CP)
                in_q[q % 3].dma_start(out=x_sb[:, sl], in_=x_d[:, sl])

            s_all = singles.tile([128, NT], f32)
            w_all = singles.tile([128, NT], f32)
            # mask tiles padded to stride 32 so 4-tile PE transposes land at
            # 32-aligned PSUM partitions (engine partition-offset rule)
            maskA = singles.tile([128, NT * 32], bf16)
            wmA = singles.tile([128, NT * K], bf16)
            maskB = singles.tile([K, N], bf16)
            aggN = singles.tile([K, C], bf16)
            dinv = singles.tile([K, 1], f32)

            # ---- Phase A (DVE/ACT) + Phase B (PE) interleaved per group ----
            # group g = tiles 8g..8g+7; phase A computes s/w/maskA/wmA; PE
            # transposes masks (4 tiles per transpose) and accumulates agg.
            with (
                tc.tile_pool(name="psAgg", bufs=1, space="PSUM") as psAggp,
                tc.tile_pool(name="psMB", bufs=2, space="PSUM") as psMBp,
            ):
                psAgg = psAggp.tile([K, CP], f32)
                for g in range(NGA):
                    pg = predsT_sb[:, g * GA * K:(g + 1) * GA * K]
                    sg = s_all[:, g * GA:(g + 1) * GA]
                    nc.vector.tensor_reduce(
                        sg,
                        pg.rearrange("p (t k) -> p t k", k=K),
                        axis=Ax.X,
                        op=Alu.max,
                    )
                    nc.scalar.activation(
                        w_all[:, g * GA:(g + 1) * GA],
                        sg,
                        mybir.ActivationFunctionType.Exp,
                    )
                    mg = maskA.rearrange("p (t kk) -> p t kk", kk=32)[
                        :, g * GA:(g + 1) * GA, 0:K
                    ]
                    nc.vector.tensor_tensor(
                        mg,
                        pg.rearrange("p (t k) -> p t k", k=K),
                        sg[:, :, None].broadcast_to([128, GA, K]),
                        Alu.is_equal,
                    )
                    nc.vector.tensor_tensor(
                        wmA[:, g * GA * K:(g + 1) * GA * K].rearrange(
                            "p (t k) -> p t k", k=K
                        ),
                        mg,
                        w_all[:, g * GA:(g + 1) * GA][:, :, None].broadcast_to(
                            [128, GA, K]
                        ),
                        Alu.mult,
                    )
                    # PE: transpose this group's masks, 4 tiles per matmul.
                    # Batches of 8 transposes share one [76, 1024] PSUM tile
                    # (4 groups of 8 tiles = 32 tiles per PSUM tile).
                    if g % 4 == 0:
                        psMB = psMBp.tile([128, 8 * TILE], bf16)
                    for half in range(2):
                        j = g * 2 + half          # batch index (0..31)
                        jj = j % 8                # slot within psMB tile
                        nc.tensor.transpose(
                            psMB[:, jj * TILE:(jj + 1) * TILE],
                            maskA[:, j * 4 * 32:(j + 1) * 4 * 32],
                            identb,
                        )
                    if g % 4 == 3:
                        # evacuate 32 tiles of transposed masks -> maskB
                        g32 = g // 4
                        mbv = maskB.rearrange(
                            "k (g2 j r p) -> k g2 j r p", j=8, r=4, p=TILE
                        )
                        for r in range(4):
                            src = psMB[r * 32:r * 32 + K].rearrange(
                                "k (j p) -> k j p", p=TILE
                            )
                            if r % 2 == 0:
                                nc.vector.tensor_copy(mbv[:, g32, :, r, :], src)
                            else:
                                nc.scalar.copy(mbv[:, g32, :, r, :], src)
                    # PE: agg matmuls for this group's tiles
                    for t in range(g * GA, (g + 1) * GA):
                        nc.tensor.matmul(
                            psAgg,
                            lhsT=wmA[:, t * K:(t + 1) * K],
                            rhs=x_sb[:, t * CP:(t + 1) * CP],
                            start=(t == 0),
                            stop=(t == NT - 1),
                        )

                # ---- Phase C: normalize (reads psAgg in place) -------------
                nc.vector.tensor_scalar(
                    dinv, psAgg[:, C:C + 1], 1e-30, None, Alu.max
                )
                nc.vector.reciprocal(dinv, dinv)
                nc.vector.tensor_scalar(
                    aggN, psAgg[:, 0:C], dinv, None, Alu.mult
                )

            # ---- Phase D: scatter out = aggN^T @ maskB ---------------------
            with (
                tc.tile_pool(name="psO", bufs=3, space="PSUM") as psOp,
                tc.tile_pool(name="ost", bufs=4) as ostp,
            ):
                out_q = [nc.scalar, nc.sync, nc.gpsimd]
                ci = 0
                for h in range(2):
                    for q in range(N // OCH):          # OCH=2048, 8 per half
                        ost = ostp.tile([128, OCH], bf16)
                        for jh in range(OCH // 1024):
                            psO = psOp.tile([128, 1024], f32)
                            for j2 in range(2):
                                nb0 = q * OCH + jh * 1024 + j2 * 512
                                nc.tensor.matmul(
                                    psO[:, j2 * 512:(j2 + 1) * 512],
                                    lhsT=aggN[:, h * 128:(h + 1) * 128],
                                    rhs=maskB[:, nb0:nb0 + 512],
                                    start=True,
                                    stop=True,
                                )
                            dst = ost[:, jh * 1024:(jh + 1) * 1024]
                            if ci % 2 == 0:
                                nc.vector.tensor_copy(dst, psO)
                            else:
                                nc.scalar.copy(dst, psO)
                            ci += 1
                        out_q[(h * 8 + q) % 3].dma_start(
                            out=o_d[h * 128:(h + 1) * 128, q * OCH:(q + 1) * OCH],
                            in_=ost,
                        )

    nc.compile()
    return nc


def _get_nc():
    if "nc" not in _CACHE:
        _CACHE["nc"] = _build_nc()
    return _CACHE["nc"]


def make_in_maps(x, preds):
    """Host-side marshaling: per batch, bf16-cast + transpose x into
    x_host[p, t, c] (c padded to 258 with [*chans, 1.0, 0.0]) and transpose
    preds into predsT_host[p, t, k] (f32, exact)."""
    x = np.asarray(x, dtype=np.float32)
    preds = np.asarray(preds, dtype=np.float32)
    in_maps = []
    for b in range(NCORES):
        xh = np.empty((TILE, NT, CP), dtype=BF16)
        # x[b]: [C, H, W] -> [C, NT, TILE]; want xh[p, t, c] = x[b][c, t, p]
        xh[:, :, :C] = x[b].reshape(C, NT, TILE).transpose(2, 1, 0)
        xh[:, :, C] = 1.0
        xh[:, :, C + 1] = 0.0
        pt = np.ascontiguousarray(
            preds[b].reshape(K, NT, TILE).transpose(2, 1, 0)
        )
        in_maps.append(
            {
                "x": xh.reshape(TILE, NT * CP),
                "predsT": pt.reshape(TILE, NT * K),
            }
        )
    return in_maps


def kernel(x, preds):
    from concourse.bass_utils import run_bass_kernel_spmd

    nc = _get_nc()
    in_maps = make_in_maps(x, preds)
    res = run_bass_kernel_spmd(nc, in_maps, list(range(NCORES)))
    out = np.stack(
        [
            np.asarray(res.results[b]["out"]).astype(np.float32).reshape(C, H, W)
            for b in range(NCORES)
        ]
    )
    return out
